# revision 30
# baseline (speedup 1.0000x reference)
"""BinHD Hamming-distance kernel for 8 Trainium2 NeuronCores.

dist[n, c] = sum_d xor(samples[n, d], classes_hv[c, d])
           = s_sum[n] + c_sum[c] - 2 * (samples @ classes_hv.T)[n, c]

Strategy (data-parallel over samples):
  - shard samples row-wise across 8 cores (1024 rows each); replicate classes.
  - per core: a [1024 x 9984] x [9984 x 1000] GEMM on the TensorEngine in
    fp8e4m3 with perf_mode=DoubleRow. Inputs are {0,1} and {0,-2} -> fp8 is
    exact; PSUM accumulates fp32 and |sums| < 2^24 -> bit-exact vs reference.
  - classes are pre-scaled by -2 so PSUM directly holds -2*cross; the epilogue
    is ONE fused DVE op per psum chunk:
      out = (psum + s_sum[p]) + cr[p, c]
    with cr[n, c] = c_sum[c] - 2 * samples[n, 9984:] @ classes[c, 9984:].T
    sent as int16 (exact: values ~5000, |R| <= 32) -- 2.07MB instead of a
    4.13MB f32 bias plane, and s_sum as a per-partition scalar AP.

DoubleRow layout: each matmul contracts K=256 via 3D APs [p, i, free] with
k = 256*t + 128*i + p (planar i-major packing in SBUF, validated on HW).

Structure (v3):
  - pass 0: m-tiles 0-3 in k-lockstep (t-major) so the one-time 9.6MB classes
    load streams straight into consumption; per at-group the 4 m-tiles are
    host-packed into ONE contiguous per-partition blob = one 2D descriptor.
  - pass 1: m-tiles 4-7 SEQUENTIALLY (m-major) so each epilogue hides under
    the next m-tile's matmuls; the last m-tile runs its two column chunks
    q-major, leaving only a single 488-col epilogue after the final matmul.
  - DMA engine queues drain in consumption order: sync=at, gpsimd=bt then
    ssum/cr, scalar=out. Engine program order + tile-pool WAR rotation is the
    real scheduler; spreading streams over queues keeps issue parallel.
"""

import sys

if "/opt/trn_rl_repo" not in sys.path:
    sys.path.insert(0, "/opt/trn_rl_repo")

import numpy as np
import ml_dtypes

N, D, C = 8192, 10000, 1000
N_CORES = 8
P = 128
TT = 39                  # k-super-tiles of 256 on the PE (covers 9984 of D)
K_MM = TT * 2 * P        # 9984
C_PAD = 1008             # classes padded 1000 -> 1008 (512 + 496 psum chunks)
NQ = 2
QSTRIDE = [512, 496]     # SBUF i-plane strides (DoubleRow: stride % 16 == 0)
QW = [512, 488]          # streamed widths; q1 streams 488 of its 496 plane
ST_B = 2 * (QSTRIDE[0] + QSTRIDE[1])   # bt elements per supertile/partition
M_SH = N // N_CORES      # 1024 sample rows per core
MT = M_SH // P           # 8 m-tiles per core
NM = 4                   # m-tiles per k-pass (2 passes x 4)

# pass-0 at/bt group sizes in supertiles: small-first ramp, steady 4
GS = [1, 1, 2, 4, 4, 4, 4, 4, 4, 4, 4, 3]
assert sum(GS) == TT
GSTART = np.cumsum([0] + GS).tolist()
NG = len(GS)
# pass-1 at groups (per m-tile; low bandwidth pressure -> coarse)
GS1 = [3, 8, 8, 8, 8, 4]
assert sum(GS1) == TT
G1START = np.cumsum([0] + GS1).tolist()
NG1 = len(GS1)

F8 = ml_dtypes.float8_e4m3

_compiled = None


def _build():
    import concourse.mybir as mybir
    from concourse import bacc
    from concourse.tile import TileContext

    nc = bacc.Bacc("TRN2", target_bir_lowering=False, debug=False)
    f8 = mybir.dt.float8e4
    f32 = mybir.dt.float32
    i16 = mybir.dt.int16
    ADD = mybir.AluOpType.add
    DR = mybir.MatmulPerfMode.DoubleRow

    # at0: [p, (g mt j i m)] pass-0 blob (4 m-tiles interleaved per group)
    at0_d = nc.declare_dram_parameter("at0", [P, NM * TT * 256], f8, isOutput=False)
    # at1: [p, (mt t i m)] pass-1 blob (per m-tile contiguous k-stream)
    at1_d = nc.declare_dram_parameter("at1", [P, NM * TT * 256], f8, isOutput=False)
    # bt: [p, (t q i n)] (-2*classes).T, per-partition contiguous
    bt_d = nc.declare_dram_parameter("bt", [P, TT * ST_B], f8, isOutput=False)
    # cr[n, c] = c_sum[c] + R[n, c] (int16-exact), R = K-remainder correction
    cr_d = nc.declare_dram_parameter("cr", [MT, P, C_PAD], i16, isOutput=False)
    ssum_d = nc.declare_dram_parameter("ssum", [P, MT], f32, isOutput=False)
    out_d = nc.declare_dram_parameter("out", [MT, P, C_PAD], f32, isOutput=True)

    def bt_rhs(btgs, t, q, lo=0, hi=None):
        if hi is None:
            hi = QW[q]
        if t == 0:
            # supertile 0 is loaded as two per-plane DMAs (separate tiles)
            # so the very first matmul gates on 129KB, not 258KB
            return btgs[0][q].rearrange("p (i n) -> p i n", i=2)[:, :, lo:hi]
        b = int(np.searchsorted(GSTART, t, side="right")) - 1
        jb = t - GSTART[b]
        qb = jb * ST_B + q * 2 * QSTRIDE[0]
        return btgs[b][
            :, qb:qb + 2 * QSTRIDE[q]
        ].rearrange("p (i n) -> p i n", i=2)[:, :, lo:hi]

    with TileContext(nc) as tc:
        with (
            tc.tile_pool(name="btp", bufs=1) as btp,
            tc.tile_pool(name="atp", bufs=1) as atp,
            tc.tile_pool(name="pp", bufs=1, space="PSUM") as pp,
            tc.tile_pool(name="op", bufs=3) as op,
            tc.tile_pool(name="crp", bufs=1) as crp,
            tc.tile_pool(name="sp", bufs=1) as sp,
        ):
            # ---- bt: whole classes matrix, split over TWO queues (gpsimd +
            # scalar, alternating groups) so the early ramp gets 2/3 of DMA
            # bandwidth against the unthrottled at flood; resident in SBUF ----
            b0a = btp.tile([P, 2 * QSTRIDE[0]], f8, tag="btg0a", name="btg0a")
            nc.gpsimd.dma_start(out=b0a, in_=bt_d[:, 0:2 * QSTRIDE[0]])
            b0b = btp.tile([P, 2 * QSTRIDE[1]], f8, tag="btg0b", name="btg0b")
            nc.scalar.dma_start(out=b0b, in_=bt_d[:, 2 * QSTRIDE[0]:ST_B])
            btgs = [(b0a, b0b)]
            for b in range(1, NG):
                btg = btp.tile(
                    [P, GS[b] * ST_B], f8, tag=f"btg{b}", name=f"btg{b}"
                )
                eng = nc.scalar if b % 2 == 1 else nc.gpsimd
                eng.dma_start(
                    out=btg, in_=bt_d[:, GSTART[b] * ST_B:GSTART[b + 1] * ST_B]
                )
                btgs.append(btg)
            ssum_t = sp.tile([P, MT], f32, tag="ssum", name="ssum")
            nc.gpsimd.dma_start(out=ssum_t, in_=ssum_d[:, :])
            # cr on gpsimd: a follower queue so the bias tiles never steal
            # early bandwidth from the at/bt ramp
            cr_ts = []
            for m in range(MT):
                cr_t = crp.tile([P, C_PAD], i16, tag=f"cr{m}", name=f"cr{m}")
                nc.gpsimd.dma_start(out=cr_t, in_=cr_d[m])
                cr_ts.append(cr_t)

            def epilogue(li, ps_li, o_dma_engine):
                cr_t = cr_ts[li]
                o = op.tile([P, C_PAD], f32)
                nc.vector.scalar_tensor_tensor(
                    o[:, 0:512], ps_li[0][:], ssum_t[:, li:li + 1],
                    cr_t[:, 0:512], ADD, ADD,
                )
                o_dma_engine.dma_start(
                    out=out_d[li, :, 0:512], in_=o[:, 0:512]
                )
                nc.vector.scalar_tensor_tensor(
                    o[:, 512:512 + QW[1]], ps_li[1][:],
                    ssum_t[:, li:li + 1], cr_t[:, 512:512 + QW[1]], ADD, ADD,
                )
                o_dma_engine.dma_start(
                    out=out_d[li, :, 512:512 + QW[1]],
                    in_=o[:, 512:512 + QW[1]],
                )

            # ================= pass 0: m-tiles 0..3, t-major =================
            # every at group gets its own resident tile: zero buffer
            # rotation, so no DMA is ever gated on PE-progress semaphores --
            # the whole input streams at full bandwidth from program start
            ags = []
            for g in range(NG):
                ag = atp.tile(
                    [P, NM * GS[g] * 256], f8, tag=f"a0g{g}", name=f"ag_p0_g{g}"
                )
                nc.sync.dma_start(
                    out=ag,
                    in_=at0_d[
                        :, NM * GSTART[g] * 256:NM * GSTART[g + 1] * 256
                    ],
                )
                ags.append(ag)
            ags1_all = []
            for li in range(NM):
                row = []
                for g in range(NG1):
                    ag = atp.tile(
                        [P, GS1[g] * 256], f8, tag=f"a1m{li}g{g}",
                        name=f"ag_p1m{li}_g{g}",
                    )
                    nc.sync.dma_start(
                        out=ag,
                        in_=at1_d[
                            :,
                            (li * TT + G1START[g]) * 256:
                            (li * TT + G1START[g + 1]) * 256,
                        ],
                    )
                    row.append(ag)
                ags1_all.append(row)

            ps = [
                [
                    pp.tile(
                        [P, QW[q]], f32, tag=f"bank{2 * li + q}",
                        name=f"ps_p0_m{li}_q{q}",
                    )
                    for q in range(NQ)
                ]
                for li in range(NM)
            ]
            for t in range(TT):
                g = int(np.searchsorted(GSTART, t, side="right")) - 1
                j = t - GSTART[g]
                for li in range(NM):
                    lhs3 = ags[g][
                        :, (li * GS[g] + j) * 256:(li * GS[g] + j + 1) * 256
                    ].rearrange("p (i m) -> p i m", i=2)
                    for q in range(NQ):
                        nc.tensor.matmul(
                            ps[li][q], lhs3, bt_rhs(btgs, t, q),
                            start=(t == 0), stop=(t == TT - 1), perf_mode=DR,
                        )
            for li in range(NM):
                epilogue(li, ps[li], nc.scalar)

            # ================= pass 1: m-tiles 4..7, m-major =================
            for li in range(NM):
                m = NM + li
                if li < NM - 1:
                    psm = [
                        pp.tile(
                            [P, QW[q]], f32, tag=f"bank{2 * li + q}",
                            name=f"ps_p1_m{li}_q{q}",
                        )
                        for q in range(NQ)
                    ]
                ags1 = ags1_all[li]

                if li < NM - 1:
                    # t-major over the two column chunks
                    for t in range(TT):
                        g = int(np.searchsorted(G1START, t, side="right")) - 1
                        j = t - G1START[g]
                        lhs3 = ags1[g][
                            :, j * 256:(j + 1) * 256
                        ].rearrange("p (i m) -> p i m", i=2)
                        for q in range(NQ):
                            nc.tensor.matmul(
                                psm[q], lhs3, bt_rhs(btgs, t, q),
                                start=(t == 0), stop=(t == TT - 1),
                                perf_mode=DR,
                            )
                else:
                    # last m-tile: chunk-major over three column chunks
                    # (512 | 244 | 244) with alternating banks, so only a
                    # 244-col epilogue trails the final matmul
                    chunks = [
                        (0, 0, 512, f"bank{2 * li}"),
                        (1, 0, 244, f"bank{2 * li + 1}"),
                        (1, 244, 488, f"bank{2 * li}"),
                    ]
                    for ci, (q, lo, hi, btag) in enumerate(chunks):
                        pc = pp.tile(
                            [P, hi - lo], f32, tag=btag,
                            name=f"ps_p1_m{li}_c{ci}",
                        )
                        for t in range(TT):
                            g = int(np.searchsorted(G1START, t, side="right")) - 1
                            j = t - G1START[g]
                            lhs3 = ags1[g][
                                :, j * 256:(j + 1) * 256
                            ].rearrange("p (i m) -> p i m", i=2)
                            nc.tensor.matmul(
                                pc, lhs3, bt_rhs(btgs, t, q, lo, hi),
                                start=(t == 0), stop=(t == TT - 1),
                                perf_mode=DR,
                            )
                        # drain this chunk immediately
                        cr_t = cr_ts[m]
                        c0 = q * 512 + lo
                        w = hi - lo
                        o = op.tile([P, w], f32)
                        nc.vector.scalar_tensor_tensor(
                            o[:, 0:w], pc[:], ssum_t[:, m:m + 1],
                            cr_t[:, c0:c0 + w], ADD, ADD,
                        )
                        nc.scalar.dma_start(
                            out=out_d[m, :, c0:c0 + w], in_=o[:, 0:w]
                        )
                if li < NM - 1:
                    epilogue(m, psm, nc.scalar)

    nc.compile()
    return nc


def _prep_inputs(samples: np.ndarray, classes_hv: np.ndarray):
    """Host-side shard + layout prep. All values stay exactly representable."""
    samples = np.ascontiguousarray(samples, dtype=np.float32)
    classes_hv = np.ascontiguousarray(classes_hv, dtype=np.float32)

    s_sum = samples.sum(axis=1, dtype=np.float32)        # [N], ints <= D
    c_sum = classes_hv.sum(axis=1, dtype=np.float32)     # [C]
    # cr[n, c] = c_sum[c] - 2 * samples[n, 9984:] @ classes[c, 9984:].T
    # (K remainder 10000 = 39*256 + 16 folded in; exact small ints -> int16)
    cr_full = np.zeros((N, C_PAD), np.float32)
    cr_full[:, :C] = c_sum[None, :]
    cr_full[:, :C] += (-2.0 * samples[:, K_MM:]) @ classes_hv[:, K_MM:].T
    cr_full = cr_full.astype(np.int16)

    # bt: (-2*classes).T [K_MM, C_PAD]; k = 256t + 128i + p -> [p, (t q i n)]
    B8 = np.zeros((K_MM, C_PAD), F8)
    B8[:, :C] = (-2.0 * classes_hv[:, :K_MM]).astype(F8).T
    b0 = (
        B8[:, :QSTRIDE[0]].reshape(TT, 2, P, QSTRIDE[0])
        .transpose(2, 0, 1, 3).reshape(P, TT, 2 * QSTRIDE[0])
    )
    b1 = (
        B8[:, QSTRIDE[0]:].reshape(TT, 2, P, QSTRIDE[1])
        .transpose(2, 0, 1, 3).reshape(P, TT, 2 * QSTRIDE[1])
    )
    bt_host = np.ascontiguousarray(
        np.concatenate([b0, b1], axis=2).reshape(P, TT * ST_B)
    )

    in_maps = []
    for c in range(N_CORES):
        rows = slice(c * M_SH, (c + 1) * M_SH)
        A8 = samples[rows, :K_MM].astype(F8).T           # [K_MM, 1024]
        # [k, n] -> [p, mt, t, i, m]  (k = 256t + 128i + p, n = 128mt + m)
        A5 = (
            A8.reshape(TT, 2, P, MT, P)                  # [t, i, p, mt, m]
            .transpose(2, 3, 0, 1, 4)                    # [p, mt, t, i, m]
        )
        # pass 0: groups of GS supertiles x 4 m-tiles interleaved
        X0 = A5[:, 0:NM]                                 # [p, 4, t, i, m]
        at0 = np.ascontiguousarray(
            np.concatenate(
                [
                    X0[:, :, GSTART[g]:GSTART[g + 1]].reshape(P, -1)
                    for g in range(NG)
                ],
                axis=1,
            )
        )
        # pass 1: per m-tile contiguous k-stream
        at1 = np.ascontiguousarray(A5[:, NM:2 * NM].reshape(P, -1))
        cr_c = np.ascontiguousarray(cr_full[rows].reshape(MT, P, C_PAD))
        ssum_c = np.ascontiguousarray(
            s_sum[rows].reshape(MT, P).T                 # [p, mt]
        )
        in_maps.append(
            {"at0": at0, "at1": at1, "bt": bt_host, "cr": cr_c, "ssum": ssum_c}
        )
    return in_maps


def _run(inputs: dict, trace: bool = False, **spmd_kwargs):
    from concourse.bass_utils import run_bass_kernel_spmd

    global _compiled
    if _compiled is None:
        _compiled = _build()

    in_maps = _prep_inputs(inputs["samples"], inputs["classes_hv"])
    res = run_bass_kernel_spmd(
        _compiled, in_maps, list(range(N_CORES)), trace=trace, **spmd_kwargs
    )
    parts = [
        res.results[c]["out"].reshape(M_SH, C_PAD)[:, :C] for c in range(N_CORES)
    ]
    out = np.concatenate(parts, axis=0).astype(np.float32)
    return out, res


def kernel(samples: np.ndarray, classes_hv: np.ndarray) -> np.ndarray:
    out, _ = _run({"samples": samples, "classes_hv": classes_hv})
    return out


# revision 31
# speedup vs baseline: 1.0966x; 1.0966x over previous
"""BinHD Hamming-distance kernel for 8 Trainium2 NeuronCores.

dist[n, c] = sum_d xor(samples[n, d], classes_hv[c, d])
           = s_sum[n] + c_sum[c] - 2 * (samples @ classes_hv.T)[n, c]

Strategy (data-parallel over samples):
  - shard samples row-wise across 8 cores (1024 rows each); replicate classes.
  - per core: a [1024 x 9984] x [9984 x 1000] GEMM on the TensorEngine in
    fp8e4m3 with perf_mode=DoubleRow. Inputs are {0,1} and {0,-2} -> fp8 is
    exact; PSUM accumulates fp32 and |sums| < 2^24 -> bit-exact vs reference.
  - classes are pre-scaled by -2 so PSUM directly holds -2*cross; the epilogue
    is ONE fused DVE op per psum chunk:
      out = (psum + s_sum[p]) + cr[p, c]
    with cr[n, c] = c_sum[c] - 2 * samples[n, 9984:] @ classes[c, 9984:].T
    sent as int16 (exact: values ~5000, |R| <= 32) -- 2.07MB instead of a
    4.13MB f32 bias plane, and s_sum as a per-partition scalar AP.

DoubleRow layout: each matmul contracts K=256 via 3D APs [p, i, free] with
k = 256*t + 128*i + p (planar i-major packing in SBUF, validated on HW).

Structure (v3):
  - pass 0: m-tiles 0-3 in k-lockstep (t-major) so the one-time 9.6MB classes
    load streams straight into consumption; per at-group the 4 m-tiles are
    host-packed into ONE contiguous per-partition blob = one 2D descriptor.
  - pass 1: m-tiles 4-7 SEQUENTIALLY (m-major) so each epilogue hides under
    the next m-tile's matmuls; the last m-tile runs its two column chunks
    q-major, leaving only a single 488-col epilogue after the final matmul.
  - DMA engine queues drain in consumption order: sync=at, gpsimd=bt then
    ssum/cr, scalar=out. Engine program order + tile-pool WAR rotation is the
    real scheduler; spreading streams over queues keeps issue parallel.
"""

import sys

if "/opt/trn_rl_repo" not in sys.path:
    sys.path.insert(0, "/opt/trn_rl_repo")

import numpy as np
import ml_dtypes

N, D, C = 8192, 10000, 1000
N_CORES = 8
P = 128
TT = 39                  # k-super-tiles of 256 on the PE (covers 9984 of D)
K_MM = TT * 2 * P        # 9984
C_PAD = 1008             # classes padded 1000 -> 1008 (512 + 496 psum chunks)
NQ = 2
QSTRIDE = [512, 496]     # SBUF i-plane strides (DoubleRow: stride % 16 == 0)
QW = [512, 488]          # streamed widths; q1 streams 488 of its 496 plane
ST_B = 2 * (QSTRIDE[0] + QSTRIDE[1])   # bt elements per supertile/partition
M_SH = N // N_CORES      # 1024 sample rows per core
MT = M_SH // P           # 8 m-tiles per core
NM = 4                   # m-tiles per k-pass (2 passes x 4)

# pass-0 at/bt group sizes in supertiles: small-first ramp, steady 4
GS = [1, 1, 2, 4, 4, 4, 4, 4, 4, 4, 4, 3]
assert sum(GS) == TT
GSTART = np.cumsum([0] + GS).tolist()
NG = len(GS)
# pass-1 at groups (per m-tile; low bandwidth pressure -> coarse)
GS1 = [3, 8, 8, 8, 8, 4]
assert sum(GS1) == TT
G1START = np.cumsum([0] + GS1).tolist()
NG1 = len(GS1)

F8 = ml_dtypes.float8_e4m3

_compiled = None


def _build():
    import concourse.mybir as mybir
    from concourse import bacc
    from concourse.tile import TileContext

    nc = bacc.Bacc("TRN2", target_bir_lowering=False, debug=False)
    f8 = mybir.dt.float8e4
    f32 = mybir.dt.float32
    i16 = mybir.dt.int16
    ADD = mybir.AluOpType.add
    DR = mybir.MatmulPerfMode.DoubleRow

    # at0: [p, (g mt j i m)] pass-0 blob (4 m-tiles interleaved per group)
    at0_d = nc.declare_dram_parameter("at0", [P, NM * TT * 256], f8, isOutput=False)
    # at1: [p, (mt t i m)] pass-1 blob (per m-tile contiguous k-stream)
    at1_d = nc.declare_dram_parameter("at1", [P, NM * TT * 256], f8, isOutput=False)
    # bt: [p, (t q i n)] (-2*classes).T, per-partition contiguous
    bt_d = nc.declare_dram_parameter("bt", [P, TT * ST_B], f8, isOutput=False)
    # cr[n, c] = c_sum[c] + R[n, c] (int16-exact), R = K-remainder correction
    cr_d = nc.declare_dram_parameter("cr", [MT, P, C_PAD], i16, isOutput=False)
    ssum_d = nc.declare_dram_parameter("ssum", [P, MT], f32, isOutput=False)
    out_d = nc.declare_dram_parameter("out", [MT, P, C_PAD], f32, isOutput=True)

    def bt_rhs(btgs, t, q, lo=0, hi=None):
        if hi is None:
            hi = QW[q]
        if t == 0:
            # supertile 0 is loaded as two per-plane DMAs (separate tiles)
            # so the very first matmul gates on 129KB, not 258KB
            return btgs[0][q].rearrange("p (i n) -> p i n", i=2)[:, :, lo:hi]
        b = int(np.searchsorted(GSTART, t, side="right")) - 1
        jb = t - GSTART[b]
        qb = jb * ST_B + q * 2 * QSTRIDE[0]
        return btgs[b][
            :, qb:qb + 2 * QSTRIDE[q]
        ].rearrange("p (i n) -> p i n", i=2)[:, :, lo:hi]

    with TileContext(nc) as tc:
        with (
            tc.tile_pool(name="btp", bufs=1) as btp,
            tc.tile_pool(name="atp", bufs=1) as atp,
            tc.tile_pool(name="pp", bufs=1, space="PSUM") as pp,
            tc.tile_pool(name="op", bufs=3) as op,
            tc.tile_pool(name="crp", bufs=1) as crp,
            tc.tile_pool(name="sp", bufs=1) as sp,
        ):
            # ---- bt: whole classes matrix; supertile 0's q0 plane rides at
            # the HEAD of the sync queue (earliest issuer) to gate the first
            # matmul on 129KB; the rest streams on gpsimd; resident in SBUF ----
            b0a = btp.tile([P, 2 * QSTRIDE[0]], f8, tag="btg0a", name="btg0a")
            nc.sync.dma_start(out=b0a, in_=bt_d[:, 0:2 * QSTRIDE[0]])
            b0b = btp.tile([P, 2 * QSTRIDE[1]], f8, tag="btg0b", name="btg0b")
            nc.gpsimd.dma_start(out=b0b, in_=bt_d[:, 2 * QSTRIDE[0]:ST_B])
            btgs = [(b0a, b0b)]
            for b in range(1, NG):
                btg = btp.tile(
                    [P, GS[b] * ST_B], f8, tag=f"btg{b}", name=f"btg{b}"
                )
                nc.gpsimd.dma_start(
                    out=btg, in_=bt_d[:, GSTART[b] * ST_B:GSTART[b + 1] * ST_B]
                )
                btgs.append(btg)
            ssum_t = sp.tile([P, MT], f32, tag="ssum", name="ssum")
            nc.gpsimd.dma_start(out=ssum_t, in_=ssum_d[:, :])
            # cr on gpsimd: a follower queue so the bias tiles never steal
            # early bandwidth from the at/bt ramp
            cr_ts = []
            for m in range(MT):
                cr_t = crp.tile([P, C_PAD], i16, tag=f"cr{m}", name=f"cr{m}")
                nc.gpsimd.dma_start(out=cr_t, in_=cr_d[m])
                cr_ts.append(cr_t)

            def epilogue(li, ps_li, o_dma_engine):
                cr_t = cr_ts[li]
                o = op.tile([P, C_PAD], f32)
                nc.vector.scalar_tensor_tensor(
                    o[:, 0:512], ps_li[0][:], ssum_t[:, li:li + 1],
                    cr_t[:, 0:512], ADD, ADD,
                )
                o_dma_engine.dma_start(
                    out=out_d[li, :, 0:512], in_=o[:, 0:512]
                )
                nc.vector.scalar_tensor_tensor(
                    o[:, 512:512 + QW[1]], ps_li[1][:],
                    ssum_t[:, li:li + 1], cr_t[:, 512:512 + QW[1]], ADD, ADD,
                )
                o_dma_engine.dma_start(
                    out=out_d[li, :, 512:512 + QW[1]],
                    in_=o[:, 512:512 + QW[1]],
                )

            # ================= pass 0: m-tiles 0..3, t-major =================
            # pass-0 at groups use a small rotation: the WAR gate paces the
            # at stream to PE consumption, leaving bt the bandwidth during
            # the ramp (an unthrottled at flood starves bt's early groups)
            ags = [None] * NG

            ps = [
                [
                    pp.tile(
                        [P, QW[q]], f32, tag=f"bank{2 * li + q}",
                        name=f"ps_p0_m{li}_q{q}",
                    )
                    for q in range(NQ)
                ]
                for li in range(NM)
            ]
            for t in range(TT):
                g = int(np.searchsorted(GSTART, t, side="right")) - 1
                j = t - GSTART[g]
                if j == 0:
                    ag = atp.tile(
                        [P, NM * GS[g] * 256], f8, tag="atg", bufs=5,
                        name=f"ag_p0_g{g}",
                    )
                    nc.sync.dma_start(
                        out=ag,
                        in_=at0_d[
                            :, NM * GSTART[g] * 256:NM * GSTART[g + 1] * 256
                        ],
                    )
                    ags[g] = ag
                for li in range(NM):
                    lhs3 = ags[g][
                        :, (li * GS[g] + j) * 256:(li * GS[g] + j + 1) * 256
                    ].rearrange("p (i m) -> p i m", i=2)
                    for q in range(NQ):
                        nc.tensor.matmul(
                            ps[li][q], lhs3, bt_rhs(btgs, t, q),
                            start=(t == 0), stop=(t == TT - 1), perf_mode=DR,
                        )
            for li in range(NM):
                epilogue(li, ps[li], nc.scalar)

            # ================= pass 1: m-tiles 4..7, m-major =================
            for li in range(NM):
                m = NM + li
                if li < NM - 1:
                    psm = [
                        pp.tile(
                            [P, QW[q]], f32, tag=f"bank{2 * li + q}",
                            name=f"ps_p1_m{li}_q{q}",
                        )
                        for q in range(NQ)
                    ]
                ags1 = []
                for g in range(NG1):
                    ag = atp.tile(
                        [P, GS1[g] * 256], f8, tag=f"a1m{li}g{g}",
                        name=f"ag_p1m{li}_g{g}",
                    )
                    nc.sync.dma_start(
                        out=ag,
                        in_=at1_d[
                            :,
                            (li * TT + G1START[g]) * 256:
                            (li * TT + G1START[g + 1]) * 256,
                        ],
                    )
                    ags1.append(ag)

                if li < NM - 1:
                    # t-major over the two column chunks
                    for t in range(TT):
                        g = int(np.searchsorted(G1START, t, side="right")) - 1
                        j = t - G1START[g]
                        lhs3 = ags1[g][
                            :, j * 256:(j + 1) * 256
                        ].rearrange("p (i m) -> p i m", i=2)
                        for q in range(NQ):
                            nc.tensor.matmul(
                                psm[q], lhs3, bt_rhs(btgs, t, q),
                                start=(t == 0), stop=(t == TT - 1),
                                perf_mode=DR,
                            )
                else:
                    # last m-tile: chunk-major over three column chunks
                    # (512 | 244 | 244) with alternating banks, so only a
                    # 244-col epilogue trails the final matmul
                    chunks = [
                        (0, 0, 512, f"bank{2 * li}"),
                        (1, 0, 244, f"bank{2 * li + 1}"),
                        (1, 244, 488, f"bank{2 * li}"),
                    ]
                    for ci, (q, lo, hi, btag) in enumerate(chunks):
                        pc = pp.tile(
                            [P, hi - lo], f32, tag=btag,
                            name=f"ps_p1_m{li}_c{ci}",
                        )
                        for t in range(TT):
                            g = int(np.searchsorted(G1START, t, side="right")) - 1
                            j = t - G1START[g]
                            lhs3 = ags1[g][
                                :, j * 256:(j + 1) * 256
                            ].rearrange("p (i m) -> p i m", i=2)
                            nc.tensor.matmul(
                                pc, lhs3, bt_rhs(btgs, t, q, lo, hi),
                                start=(t == 0), stop=(t == TT - 1),
                                perf_mode=DR,
                            )
                        # drain this chunk immediately
                        cr_t = cr_ts[m]
                        c0 = q * 512 + lo
                        w = hi - lo
                        o = op.tile([P, w], f32)
                        nc.vector.scalar_tensor_tensor(
                            o[:, 0:w], pc[:], ssum_t[:, m:m + 1],
                            cr_t[:, c0:c0 + w], ADD, ADD,
                        )
                        nc.scalar.dma_start(
                            out=out_d[m, :, c0:c0 + w], in_=o[:, 0:w]
                        )
                if li < NM - 1:
                    epilogue(m, psm, nc.scalar)

    nc.compile()
    return nc


def _prep_inputs(samples: np.ndarray, classes_hv: np.ndarray):
    """Host-side shard + layout prep. All values stay exactly representable."""
    samples = np.ascontiguousarray(samples, dtype=np.float32)
    classes_hv = np.ascontiguousarray(classes_hv, dtype=np.float32)

    s_sum = samples.sum(axis=1, dtype=np.float32)        # [N], ints <= D
    c_sum = classes_hv.sum(axis=1, dtype=np.float32)     # [C]
    # cr[n, c] = c_sum[c] - 2 * samples[n, 9984:] @ classes[c, 9984:].T
    # (K remainder 10000 = 39*256 + 16 folded in; exact small ints -> int16)
    cr_full = np.zeros((N, C_PAD), np.float32)
    cr_full[:, :C] = c_sum[None, :]
    cr_full[:, :C] += (-2.0 * samples[:, K_MM:]) @ classes_hv[:, K_MM:].T
    cr_full = cr_full.astype(np.int16)

    # bt: (-2*classes).T [K_MM, C_PAD]; k = 256t + 128i + p -> [p, (t q i n)]
    B8 = np.zeros((K_MM, C_PAD), F8)
    B8[:, :C] = (-2.0 * classes_hv[:, :K_MM]).astype(F8).T
    b0 = (
        B8[:, :QSTRIDE[0]].reshape(TT, 2, P, QSTRIDE[0])
        .transpose(2, 0, 1, 3).reshape(P, TT, 2 * QSTRIDE[0])
    )
    b1 = (
        B8[:, QSTRIDE[0]:].reshape(TT, 2, P, QSTRIDE[1])
        .transpose(2, 0, 1, 3).reshape(P, TT, 2 * QSTRIDE[1])
    )
    bt_host = np.ascontiguousarray(
        np.concatenate([b0, b1], axis=2).reshape(P, TT * ST_B)
    )

    in_maps = []
    for c in range(N_CORES):
        rows = slice(c * M_SH, (c + 1) * M_SH)
        A8 = samples[rows, :K_MM].astype(F8).T           # [K_MM, 1024]
        # [k, n] -> [p, mt, t, i, m]  (k = 256t + 128i + p, n = 128mt + m)
        A5 = (
            A8.reshape(TT, 2, P, MT, P)                  # [t, i, p, mt, m]
            .transpose(2, 3, 0, 1, 4)                    # [p, mt, t, i, m]
        )
        # pass 0: groups of GS supertiles x 4 m-tiles interleaved
        X0 = A5[:, 0:NM]                                 # [p, 4, t, i, m]
        at0 = np.ascontiguousarray(
            np.concatenate(
                [
                    X0[:, :, GSTART[g]:GSTART[g + 1]].reshape(P, -1)
                    for g in range(NG)
                ],
                axis=1,
            )
        )
        # pass 1: per m-tile contiguous k-stream
        at1 = np.ascontiguousarray(A5[:, NM:2 * NM].reshape(P, -1))
        cr_c = np.ascontiguousarray(cr_full[rows].reshape(MT, P, C_PAD))
        ssum_c = np.ascontiguousarray(
            s_sum[rows].reshape(MT, P).T                 # [p, mt]
        )
        in_maps.append(
            {"at0": at0, "at1": at1, "bt": bt_host, "cr": cr_c, "ssum": ssum_c}
        )
    return in_maps


def _run(inputs: dict, trace: bool = False, **spmd_kwargs):
    from concourse.bass_utils import run_bass_kernel_spmd

    global _compiled
    if _compiled is None:
        _compiled = _build()

    in_maps = _prep_inputs(inputs["samples"], inputs["classes_hv"])
    res = run_bass_kernel_spmd(
        _compiled, in_maps, list(range(N_CORES)), trace=trace, **spmd_kwargs
    )
    parts = [
        res.results[c]["out"].reshape(M_SH, C_PAD)[:, :C] for c in range(N_CORES)
    ]
    out = np.concatenate(parts, axis=0).astype(np.float32)
    return out, res


def kernel(samples: np.ndarray, classes_hv: np.ndarray) -> np.ndarray:
    out, _ = _run({"samples": samples, "classes_hv": classes_hv})
    return out


# revision 32
# speedup vs baseline: 1.1118x; 1.0138x over previous
"""BinHD Hamming-distance kernel for 8 Trainium2 NeuronCores.

dist[n, c] = sum_d xor(samples[n, d], classes_hv[c, d])
           = s_sum[n] + c_sum[c] - 2 * (samples @ classes_hv.T)[n, c]

Strategy (data-parallel over samples):
  - shard samples row-wise across 8 cores (1024 rows each); replicate classes.
  - per core: a [1024 x 9984] x [9984 x 1000] GEMM on the TensorEngine in
    fp8e4m3 with perf_mode=DoubleRow. Inputs are {0,1} and {0,-2} -> fp8 is
    exact; PSUM accumulates fp32 and |sums| < 2^24 -> bit-exact vs reference.
  - classes are pre-scaled by -2 so PSUM directly holds -2*cross; the epilogue
    is ONE fused DVE op per psum chunk:
      out = (psum + s_sum[p]) + cr[p, c]
    with cr[n, c] = c_sum[c] - 2 * samples[n, 9984:] @ classes[c, 9984:].T
    sent as int16 (exact: values ~5000, |R| <= 32) -- 2.07MB instead of a
    4.13MB f32 bias plane, and s_sum as a per-partition scalar AP.

DoubleRow layout: each matmul contracts K=256 via 3D APs [p, i, free] with
k = 256*t + 128*i + p (planar i-major packing in SBUF, validated on HW).

Structure (v3):
  - pass 0: m-tiles 0-3 in k-lockstep (t-major) so the one-time 9.6MB classes
    load streams straight into consumption; per at-group the 4 m-tiles are
    host-packed into ONE contiguous per-partition blob = one 2D descriptor.
  - pass 1: m-tiles 4-7 SEQUENTIALLY (m-major) so each epilogue hides under
    the next m-tile's matmuls; the last m-tile runs its two column chunks
    q-major, leaving only a single 488-col epilogue after the final matmul.
  - DMA engine queues drain in consumption order: sync=at, gpsimd=bt then
    ssum/cr, scalar=out. Engine program order + tile-pool WAR rotation is the
    real scheduler; spreading streams over queues keeps issue parallel.
"""

import sys

if "/opt/trn_rl_repo" not in sys.path:
    sys.path.insert(0, "/opt/trn_rl_repo")

import numpy as np
import ml_dtypes

N, D, C = 8192, 10000, 1000
N_CORES = 8
P = 128
TT = 39                  # k-super-tiles of 256 on the PE (covers 9984 of D)
K_MM = TT * 2 * P        # 9984
C_PAD = 1008             # classes padded 1000 -> 1008 (512 + 496 psum chunks)
NQ = 2
QSTRIDE = [512, 496]     # SBUF i-plane strides (DoubleRow: stride % 16 == 0)
QW = [512, 488]          # streamed widths; q1 streams 488 of its 496 plane
ST_B = 2 * (QSTRIDE[0] + QSTRIDE[1])   # bt elements per supertile/partition
M_SH = N // N_CORES      # 1024 sample rows per core
MT = M_SH // P           # 8 m-tiles per core
NM = 4                   # m-tiles per k-pass (2 passes x 4)

# pass-0 at/bt group sizes in supertiles: small-first ramp, steady 4
GS = [1, 1, 2, 4, 4, 4, 4, 4, 4, 4, 4, 3]
assert sum(GS) == TT
GSTART = np.cumsum([0] + GS).tolist()
NG = len(GS)
# pass-1 at groups (per m-tile; low bandwidth pressure -> coarse)
GS1 = [3, 8, 8, 8, 8, 4]
assert sum(GS1) == TT
G1START = np.cumsum([0] + GS1).tolist()
NG1 = len(GS1)

F8 = ml_dtypes.float8_e4m3

_compiled = None


def _build():
    import concourse.mybir as mybir
    from concourse import bacc
    from concourse.tile import TileContext

    nc = bacc.Bacc("TRN2", target_bir_lowering=False, debug=False)
    f8 = mybir.dt.float8e4
    f32 = mybir.dt.float32
    i16 = mybir.dt.int16
    ADD = mybir.AluOpType.add
    DR = mybir.MatmulPerfMode.DoubleRow

    # at0: [p, (g mt j i m)] pass-0 blob (4 m-tiles interleaved per group)
    at0_d = nc.declare_dram_parameter("at0", [P, NM * TT * 256], f8, isOutput=False)
    # at1: [p, (mt t i m)] pass-1 blob (per m-tile contiguous k-stream)
    at1_d = nc.declare_dram_parameter("at1", [P, NM * TT * 256], f8, isOutput=False)
    # bt: [p, (t q i n)] (-2*classes).T, per-partition contiguous
    bt_d = nc.declare_dram_parameter("bt", [P, TT * ST_B], f8, isOutput=False)
    # cr[n, c] = c_sum[c] + R[n, c] (int16-exact), R = K-remainder correction
    cr_d = nc.declare_dram_parameter("cr", [MT, P, C_PAD], i16, isOutput=False)
    ssum_d = nc.declare_dram_parameter("ssum", [P, MT], f32, isOutput=False)
    out_d = nc.declare_dram_parameter("out", [MT, P, C_PAD], f32, isOutput=True)

    def bt_rhs(btgs, t, q, lo=0, hi=None):
        if hi is None:
            hi = QW[q]
        b = int(np.searchsorted(GSTART, t, side="right")) - 1
        jb = t - GSTART[b]
        qb = jb * ST_B + q * 2 * QSTRIDE[0]
        return btgs[b][
            :, qb:qb + 2 * QSTRIDE[q]
        ].rearrange("p (i n) -> p i n", i=2)[:, :, lo:hi]

    with TileContext(nc) as tc:
        with (
            tc.tile_pool(name="btp", bufs=1) as btp,
            tc.tile_pool(name="atp", bufs=1) as atp,
            tc.tile_pool(name="pp", bufs=1, space="PSUM") as pp,
            tc.tile_pool(name="op", bufs=3) as op,
            tc.tile_pool(name="crp", bufs=1) as crp,
            tc.tile_pool(name="sp", bufs=1) as sp,
        ):
            # ---- bt: whole classes matrix, queued in consumption order on
            # gpsimd; stays resident in SBUF for both passes ----
            btgs = []
            for b in range(NG):
                btg = btp.tile(
                    [P, GS[b] * ST_B], f8, tag=f"btg{b}", name=f"btg{b}"
                )
                nc.gpsimd.dma_start(
                    out=btg, in_=bt_d[:, GSTART[b] * ST_B:GSTART[b + 1] * ST_B]
                )
                btgs.append(btg)
            ssum_t = sp.tile([P, MT], f32, tag="ssum", name="ssum")
            nc.gpsimd.dma_start(out=ssum_t, in_=ssum_d[:, :])
            # cr on gpsimd: a follower queue so the bias tiles never steal
            # early bandwidth from the at/bt ramp
            cr_ts = []
            for m in range(MT):
                cr_t = crp.tile([P, C_PAD], i16, tag=f"cr{m}", name=f"cr{m}")
                nc.gpsimd.dma_start(out=cr_t, in_=cr_d[m])
                cr_ts.append(cr_t)

            def epilogue(li, ps_li, o_dma_engine):
                cr_t = cr_ts[li]
                o = op.tile([P, C_PAD], f32)
                nc.vector.scalar_tensor_tensor(
                    o[:, 0:512], ps_li[0][:], ssum_t[:, li:li + 1],
                    cr_t[:, 0:512], ADD, ADD,
                )
                o_dma_engine.dma_start(
                    out=out_d[li, :, 0:512], in_=o[:, 0:512]
                )
                nc.vector.scalar_tensor_tensor(
                    o[:, 512:512 + QW[1]], ps_li[1][:],
                    ssum_t[:, li:li + 1], cr_t[:, 512:512 + QW[1]], ADD, ADD,
                )
                o_dma_engine.dma_start(
                    out=out_d[li, :, 512:512 + QW[1]],
                    in_=o[:, 512:512 + QW[1]],
                )

            # ================= pass 0: m-tiles 0..3, t-major =================
            # pass-0 at groups use a small rotation: the WAR gate paces the
            # at stream to PE consumption, leaving bt the bandwidth during
            # the ramp (an unthrottled at flood starves bt's early groups)
            ags = [None] * NG

            ps = [
                [
                    pp.tile(
                        [P, QW[q]], f32, tag=f"bank{2 * li + q}",
                        name=f"ps_p0_m{li}_q{q}",
                    )
                    for q in range(NQ)
                ]
                for li in range(NM)
            ]
            for t in range(TT):
                g = int(np.searchsorted(GSTART, t, side="right")) - 1
                j = t - GSTART[g]
                if j == 0:
                    ag = atp.tile(
                        [P, NM * GS[g] * 256], f8, tag="atg", bufs=4,
                        name=f"ag_p0_g{g}",
                    )
                    nc.sync.dma_start(
                        out=ag,
                        in_=at0_d[
                            :, NM * GSTART[g] * 256:NM * GSTART[g + 1] * 256
                        ],
                    )
                    ags[g] = ag
                for li in range(NM):
                    lhs3 = ags[g][
                        :, (li * GS[g] + j) * 256:(li * GS[g] + j + 1) * 256
                    ].rearrange("p (i m) -> p i m", i=2)
                    for q in range(NQ):
                        nc.tensor.matmul(
                            ps[li][q], lhs3, bt_rhs(btgs, t, q),
                            start=(t == 0), stop=(t == TT - 1), perf_mode=DR,
                        )
            for li in range(NM):
                epilogue(li, ps[li], nc.scalar)

            # ================= pass 1: m-tiles 4..7, m-major =================
            for li in range(NM):
                m = NM + li
                if li < NM - 1:
                    psm = [
                        pp.tile(
                            [P, QW[q]], f32, tag=f"bank{2 * li + q}",
                            name=f"ps_p1_m{li}_q{q}",
                        )
                        for q in range(NQ)
                    ]
                ags1 = []
                for g in range(NG1):
                    ag = atp.tile(
                        [P, GS1[g] * 256], f8, tag="atg", bufs=4,
                        name=f"ag_p1m{li}_g{g}",
                    )
                    nc.sync.dma_start(
                        out=ag,
                        in_=at1_d[
                            :,
                            (li * TT + G1START[g]) * 256:
                            (li * TT + G1START[g + 1]) * 256,
                        ],
                    )
                    ags1.append(ag)

                if li < NM - 1:
                    # t-major over the two column chunks
                    for t in range(TT):
                        g = int(np.searchsorted(G1START, t, side="right")) - 1
                        j = t - G1START[g]
                        lhs3 = ags1[g][
                            :, j * 256:(j + 1) * 256
                        ].rearrange("p (i m) -> p i m", i=2)
                        for q in range(NQ):
                            nc.tensor.matmul(
                                psm[q], lhs3, bt_rhs(btgs, t, q),
                                start=(t == 0), stop=(t == TT - 1),
                                perf_mode=DR,
                            )
                else:
                    # last m-tile: q-major so chunk 0's epilogue hides under
                    # chunk 1's k-loop; only the 488-col epilogue is exposed
                    psm = [
                        pp.tile(
                            [P, QW[q]], f32, tag=f"bank{2 * li + q}",
                            name=f"ps_p1_m{li}_q{q}",
                        )
                        for q in range(NQ)
                    ]
                    for q in range(NQ):
                        for t in range(TT):
                            g = int(np.searchsorted(G1START, t, side="right")) - 1
                            j = t - G1START[g]
                            lhs3 = ags1[g][
                                :, j * 256:(j + 1) * 256
                            ].rearrange("p (i m) -> p i m", i=2)
                            nc.tensor.matmul(
                                psm[q], lhs3, bt_rhs(btgs, t, q),
                                start=(t == 0), stop=(t == TT - 1),
                                perf_mode=DR,
                            )
                        # drain this chunk immediately
                        cr_t = cr_ts[m]
                        o = op.tile([P, QW[q]], f32)
                        nc.vector.scalar_tensor_tensor(
                            o[:, 0:QW[q]], psm[q][:], ssum_t[:, m:m + 1],
                            cr_t[:, q * 512:q * 512 + QW[q]], ADD, ADD,
                        )
                        nc.scalar.dma_start(
                            out=out_d[m, :, q * 512:q * 512 + QW[q]],
                            in_=o[:, 0:QW[q]],
                        )
                if li < NM - 1:
                    epilogue(m, psm, nc.scalar)

    nc.compile()
    return nc


def _prep_inputs(samples: np.ndarray, classes_hv: np.ndarray):
    """Host-side shard + layout prep. All values stay exactly representable."""
    samples = np.ascontiguousarray(samples, dtype=np.float32)
    classes_hv = np.ascontiguousarray(classes_hv, dtype=np.float32)

    s_sum = samples.sum(axis=1, dtype=np.float32)        # [N], ints <= D
    c_sum = classes_hv.sum(axis=1, dtype=np.float32)     # [C]
    # cr[n, c] = c_sum[c] - 2 * samples[n, 9984:] @ classes[c, 9984:].T
    # (K remainder 10000 = 39*256 + 16 folded in; exact small ints -> int16)
    cr_full = np.zeros((N, C_PAD), np.float32)
    cr_full[:, :C] = c_sum[None, :]
    cr_full[:, :C] += (-2.0 * samples[:, K_MM:]) @ classes_hv[:, K_MM:].T
    cr_full = cr_full.astype(np.int16)

    # bt: (-2*classes).T [K_MM, C_PAD]; k = 256t + 128i + p -> [p, (t q i n)]
    B8 = np.zeros((K_MM, C_PAD), F8)
    B8[:, :C] = (-2.0 * classes_hv[:, :K_MM]).astype(F8).T
    b0 = (
        B8[:, :QSTRIDE[0]].reshape(TT, 2, P, QSTRIDE[0])
        .transpose(2, 0, 1, 3).reshape(P, TT, 2 * QSTRIDE[0])
    )
    b1 = (
        B8[:, QSTRIDE[0]:].reshape(TT, 2, P, QSTRIDE[1])
        .transpose(2, 0, 1, 3).reshape(P, TT, 2 * QSTRIDE[1])
    )
    bt_host = np.ascontiguousarray(
        np.concatenate([b0, b1], axis=2).reshape(P, TT * ST_B)
    )

    in_maps = []
    for c in range(N_CORES):
        rows = slice(c * M_SH, (c + 1) * M_SH)
        A8 = samples[rows, :K_MM].astype(F8).T           # [K_MM, 1024]
        # [k, n] -> [p, mt, t, i, m]  (k = 256t + 128i + p, n = 128mt + m)
        A5 = (
            A8.reshape(TT, 2, P, MT, P)                  # [t, i, p, mt, m]
            .transpose(2, 3, 0, 1, 4)                    # [p, mt, t, i, m]
        )
        # pass 0: groups of GS supertiles x 4 m-tiles interleaved
        X0 = A5[:, 0:NM]                                 # [p, 4, t, i, m]
        at0 = np.ascontiguousarray(
            np.concatenate(
                [
                    X0[:, :, GSTART[g]:GSTART[g + 1]].reshape(P, -1)
                    for g in range(NG)
                ],
                axis=1,
            )
        )
        # pass 1: per m-tile contiguous k-stream
        at1 = np.ascontiguousarray(A5[:, NM:2 * NM].reshape(P, -1))
        cr_c = np.ascontiguousarray(cr_full[rows].reshape(MT, P, C_PAD))
        ssum_c = np.ascontiguousarray(
            s_sum[rows].reshape(MT, P).T                 # [p, mt]
        )
        in_maps.append(
            {"at0": at0, "at1": at1, "bt": bt_host, "cr": cr_c, "ssum": ssum_c}
        )
    return in_maps


def _run(inputs: dict, trace: bool = False, **spmd_kwargs):
    from concourse.bass_utils import run_bass_kernel_spmd

    global _compiled
    if _compiled is None:
        _compiled = _build()

    in_maps = _prep_inputs(inputs["samples"], inputs["classes_hv"])
    res = run_bass_kernel_spmd(
        _compiled, in_maps, list(range(N_CORES)), trace=trace, **spmd_kwargs
    )
    parts = [
        res.results[c]["out"].reshape(M_SH, C_PAD)[:, :C] for c in range(N_CORES)
    ]
    out = np.concatenate(parts, axis=0).astype(np.float32)
    return out, res


def kernel(samples: np.ndarray, classes_hv: np.ndarray) -> np.ndarray:
    out, _ = _run({"samples": samples, "classes_hv": classes_hv})
    return out
